# revision 12
# baseline (speedup 1.0000x reference)
"""MidMaxPooling2D Trainium2 kernel.

Full input x: [16, 256, 256, 64] f32.  Output: [16, 128, 128, 64] f32.
out = 0.5 * max4 + 0.5 * relu(mid), where over each 2x2 window (stride 2)
max4 is the window max and mid is the 2nd-smallest of the 4 values.

Sharding: pure data parallelism over batch - 2 batches per core on 8 cores.

Per-core program (SPMD, identical on all cores).  Measured constraints that
shaped this design (TRN2, f32):
  - DVE tensor_tensor = 2292 ns per 2048-wide op; strided APs are FREE.
  - GpSimd(Pool) shares SBUF ports with DVE: running it concurrently
    degrades DVE 2.5x -> Pool is a net NEGATIVE; banned.
  - ACT (scalar engine) runs fully parallel to DVE, 2000 ns/op.
  - PE fp32 identity-matmul: ~1.26 us per 512-wide logical matmul
    (2 HW passes + ldweights), exact for +-I / 0.5*I weights.
  - DMA floor for this traffic (42 MB/core) ~ 111 us.

  partition dim = row-pair (128); E = even rows, O = odd rows (contiguous
  16 KB/partition loads); *_e / *_o = w-parity strided views.

  DVE : S = max(E,O) [4096], sm_e = min(Ee,Oe), sm_o = min(Eo,Oo),
        x4 = max(S_e,S_o), n = min(S_e,S_o), m = max(sm_e,sm_o),
        v1 = min(m,n)                       (~18.2 us/chunk -> bottleneck)
  ACT : rv = relu(v1)
  PE  : psum_out = 0.5I @ x4 + 0.5I @ rv   (blend, PSUM double-buffered)
  DMA : E,O in; out straight from PSUM
"""

import numpy as np

import concourse.bass as bass
import concourse.bacc as bacc
import concourse.tile as tile
from concourse import mybir
from concourse.bass_utils import run_bass_kernel_spmd

N_CORES = 8
B_PER_CORE = 2
H, W, C = 256, 256, 64
HO, WO = H // 2, W // 2
P = 128                      # partitions = row-pair count
WC_IN = 32                   # input w columns per chunk
FD_IN = WC_IN * C            # 2048
FD_OUT = FD_IN // 2          # 1024
N_CHUNKS = W // WC_IN        # 8 per batch
MM_N = 512                   # one PSUM bank of fp32

F32 = mybir.dt.float32
ALU = mybir.AluOpType
RELU = mybir.ActivationFunctionType.Relu


def _build_program():
    nc = bacc.Bacc(
        "TRN2", target_bir_lowering=False, debug=False, num_devices=N_CORES
    )
    x = nc.dram_tensor(
        "x", [B_PER_CORE, H, W, C], F32, kind="ExternalInput"
    ).ap()
    wh = nc.dram_tensor("wh", [P, P], F32, kind="ExternalInput").ap()  # 0.5*I
    out = nc.dram_tensor(
        "out", [B_PER_CORE, HO, WO, C], F32, kind="ExternalOutput"
    ).ap()

    xr = x.rearrange("b (h p) w c -> b p h (w c)", p=2)
    outr = out.rearrange("b h w c -> b h (w c)")

    with tile.TileContext(nc) as tc:
        with (
            tc.tile_pool(name="pw", bufs=1) as pw,
            tc.tile_pool(name="pin", bufs=3) as pin,
            tc.tile_pool(name="pmid", bufs=3) as pmid,
            tc.tile_pool(name="ppsum", bufs=3, space="PSUM") as ppsum,
        ):
            w_half = pw.tile([P, P], F32, tag="w_half")
            nc.sync.dma_start(w_half[:], wh[:])

            for b in range(B_PER_CORE):
                for ci in range(N_CHUNKS):
                    lo = ci * FD_IN
                    e = pin.tile([P, FD_IN], F32, tag="E")
                    o = pin.tile([P, FD_IN], F32, tag="O")
                    nc.sync.dma_start(e[:], xr[b, 0, :, lo : lo + FD_IN])
                    nc.sync.dma_start(o[:], xr[b, 1, :, lo : lo + FD_IN])

                    # w-parity strided views [p, w2, c]
                    ev = e[:].rearrange("p (w q c) -> p w q c", q=2, c=C)
                    ov = o[:].rearrange("p (w q c) -> p w q c", q=2, c=C)
                    ee, eo = ev[:, :, 0, :], ev[:, :, 1, :]
                    oe, oo = ov[:, :, 0, :], ov[:, :, 1, :]

                    s = pmid.tile([P, FD_IN], F32, tag="S")
                    nc.vector.tensor_tensor(s[:], e[:], o[:], ALU.max)
                    sv = s[:].rearrange("p (w q c) -> p w q c", q=2, c=C)
                    se, so_ = sv[:, :, 0, :], sv[:, :, 1, :]

                    # x4 first: unblocks the PE blend's first matmul early
                    x4 = pmid.tile([P, FD_OUT], F32, tag="x4")
                    x4v = x4[:].rearrange("p (w c) -> p w c", c=C)
                    nc.vector.tensor_tensor(x4v, se, so_, ALU.max)

                    sme = pmid.tile([P, FD_OUT], F32, tag="sme")
                    smo = pmid.tile([P, FD_OUT], F32, tag="smo")
                    smev = sme[:].rearrange("p (w c) -> p w c", c=C)
                    smov = smo[:].rearrange("p (w c) -> p w c", c=C)
                    nc.vector.tensor_tensor(smev, ee, oe, ALU.min)
                    nc.vector.tensor_tensor(smov, eo, oo, ALU.min)

                    n = pmid.tile([P, FD_OUT], F32, tag="n")
                    m = pmid.tile([P, FD_OUT], F32, tag="m")
                    nv = n[:].rearrange("p (w c) -> p w c", c=C)
                    nc.vector.tensor_tensor(m[:], sme[:], smo[:], ALU.max)
                    nc.vector.tensor_tensor(nv, se, so_, ALU.min)
                    nc.vector.tensor_tensor(n[:], m[:], n[:], ALU.min)

                    # ACT: rv = relu(v1)   (in place over n)
                    nc.scalar.activation(n[:], n[:], RELU)

                    # PE blend: psum = 0.5I @ x4 + 0.5I @ rv
                    ps = ppsum.tile([P, FD_OUT], F32, tag="po")
                    for j in range(FD_OUT // MM_N):
                        sl = slice(j * MM_N, (j + 1) * MM_N)
                        nc.tensor.matmul(
                            ps[:, sl], w_half[:], x4[:, sl], start=True, stop=False
                        )
                        nc.tensor.matmul(
                            ps[:, sl], w_half[:], n[:, sl], start=False, stop=True
                        )

                    # ACT: copy blend out of PSUM (DMA cannot read PSUM)
                    res = pmid.tile([P, FD_OUT], F32, tag="res")
                    nc.scalar.copy(res[:], ps[:])

                    olo = ci * FD_OUT
                    nc.sync.dma_start(outr[b, :, olo : olo + FD_OUT], res[:])

    nc.compile()
    return nc


_NC = None


def _get_nc():
    global _NC
    if _NC is None:
        _NC = _build_program()
    return _NC


_WH = None


def _in_maps(x):
    global _WH
    if _WH is None:
        _WH = (0.5 * np.eye(P)).astype(np.float32)
    return [
        {
            "x": np.ascontiguousarray(x[c * B_PER_CORE : (c + 1) * B_PER_CORE]),
            "wh": _WH,
        }
        for c in range(N_CORES)
    ]


def _run(x, trace=False):
    nc = _get_nc()
    res = run_bass_kernel_spmd(
        nc, _in_maps(x), core_ids=list(range(N_CORES)), trace=trace
    )
    full = np.concatenate([res.results[c]["out"] for c in range(N_CORES)], axis=0)
    return full, res


def kernel(x):
    x = np.asarray(x, dtype=np.float32)
    full, _ = _run(x, trace=False)
    return full


def _install_ntff_hook():
    """The image's antenv lacks axon_hooks; synthesize it and register the
    ctypes NTFF profiling hook so trace=True yields exec_time_ns."""
    import sys
    import types

    try:
        from antenv.axon_hooks import get_axon_ntff_profile_hook

        if get_axon_ntff_profile_hook() is not None:
            return
    except ImportError:
        pass
    import antenv

    mod = types.ModuleType("antenv.axon_hooks")
    holder = {}
    mod.set_axon_ntff_profile_hook = lambda h: holder.__setitem__("h", h)
    mod.get_axon_ntff_profile_hook = lambda: holder.get("h")
    sys.modules["antenv.axon_hooks"] = mod
    antenv.axon_hooks = mod
    from trn_agent_boot.trn_boot import _ntff_profile_via_ctypes

    mod.set_axon_ntff_profile_hook(
        _ntff_profile_via_ctypes("/opt/axon/libaxon_pjrt.so")
    )


def run_traced(x):
    """Returns (output, BassKernelResults with exec_time_ns) - for test.py."""
    _install_ntff_hook()
    x = np.asarray(x, dtype=np.float32)
    return _run(x, trace=True)
